# revision 1
# baseline (speedup 1.0000x reference)
"""Gated multi-head self-attention on 8 Trainium2 NeuronCores.

Sharding: 16 heads / 8 cores = 2 heads per core. Each core computes its two
heads end-to-end (QKV projection, attention, per-head norm, output
projection) and writes a partial [B*T, D] output; the host sums the 8
partials and adds the head-summed output bias.

Fast path (zero mask), per core, heads packed on partitions (p = h*64 + d),
bf16 storage / f32 PSUM accumulation:

  QT/KT[128, 4096]   = W_{q,k}.T @ x.T + b          (bf16)
  V'[tok, 2, 64]     = x @ W_v  (direct matmul, token-partition layout)
  S[128, 1024]       = K.T Q per 128-key block, 2 heads on PE quadrants
  P                  = exp(0.125 * S)               (bf16, no max subtraction:
                                                     scores are O(1))
  [out^T; d] += V'.T P   (col 64 of V' is ones => row 64 = softmax denom)
  osm                = out^T * bcast(1/d) + b_v     (softmax weights sum to 1
                                                     => V bias is additive)
  denom_h            = max(mean_t ||osm[:, t]||, 1e-5)
  out               += osm.T @ (W_o * g/16 * (1/denom_h))  (1/denom folded
                                                            into W_o rows)

Batch-1 QKV projection is emitted interleaved with batch-0 attention so the
PE fills the slack under the Act-engine exp stream (the roofline of this
kernel). The projection pipeline alternates PSUM->SBUF copies between DVE
and Pool and streams DMA out of 4 rotating buffers.
"""

import sys

sys.path.insert(0, "/opt/trn_rl_repo")

import contextlib

import numpy as np
import ml_dtypes

import concourse.bacc as bacc
import concourse.mybir as mybir
import concourse.tile as tile
from concourse.bass_utils import run_bass_kernel_spmd
from concourse.masks import make_identity

f32 = mybir.dt.float32
f32r = mybir.dt.float32r
bf16 = mybir.dt.bfloat16
AF = mybir.ActivationFunctionType
ALU = mybir.AluOpType

B, T, D, H, HD = 2, 2048, 1024, 16, 64
NCORES = 8
HPC = H // NCORES  # heads per core = 2
NT = B * T         # 4096 tokens
NJ = T // 128      # 16 key blocks per batch
SCALE = 1.0 / np.sqrt(HD)  # 0.125

_BUILD_CACHE = {}
INTERLEAVE_QKV = False


def _build(with_mask: bool, repeat: int = 1):
    if not with_mask:
        return _build_fast(repeat=repeat)
    return _build_masked(repeat=repeat)


# ---------------------------------------------------------------------------
# fast path: zero attention mask
# ---------------------------------------------------------------------------

def _build_fast(repeat: int = 1, debug: bool = False, parts: str = "full"):
    nc = bacc.Bacc(None, target_bir_lowering=False)

    xbf = nc.declare_dram_parameter("xbf", [D, NT], bf16, isOutput=False)
    wqkv = nc.declare_dram_parameter("wqkv", [3, 8, 128, 128], bf16,
                                     isOutput=False)
    bqk = nc.declare_dram_parameter("bqk", [2, 128], f32, isOutput=False)
    bvp = nc.declare_dram_parameter("bvp", [64, 2], f32, isOutput=False)
    wo = nc.declare_dram_parameter("wo", [128, D], bf16, isOutput=False)
    onesrow_d = nc.declare_dram_parameter("onesrow_d", [65, 64], f32r,
                                          isOutput=False)
    sel65_d = nc.declare_dram_parameter("sel65_d", [65, 128], f32,
                                        isOutput=False)
    outp = nc.declare_dram_parameter("outp", [NT, D], bf16, isOutput=True)
    if debug:
        dQT = nc.declare_dram_parameter("dQT", [128, NT], bf16, isOutput=True)
        dKT = nc.declare_dram_parameter("dKT", [128, NT], bf16, isOutput=True)
        dVp = nc.declare_dram_parameter("dVp", [128, B * NJ, HPC, 66], bf16,
                                        isOutput=True)
        dosm = nc.declare_dram_parameter("dosm", [128, NT], bf16,
                                         isOutput=True)
        dnsq = nc.declare_dram_parameter("dnsq", [65, NT], f32, isOutput=True)
        dwos = nc.declare_dram_parameter("dwos", [128, D], bf16, isOutput=True)

    with tile.TileContext(nc) as tc, contextlib.ExitStack() as ctx:
        wp = ctx.enter_context(tc.tile_pool(name="wp", bufs=1))
        pp = ctx.enter_context(tc.tile_pool(name="pp", bufs=4))
        cc = ctx.enter_context(tc.tile_pool(name="cc", bufs=2))
        sc = ctx.enter_context(tc.tile_pool(name="sc", bufs=2))
        ob = ctx.enter_context(tc.tile_pool(name="ob", bufs=6))
        psum = ctx.enter_context(tc.tile_pool(name="psum", bufs=3, space="PSUM"))

        # ---- constants / weights ----
        wqkv_sb = wp.tile([128, 3, 8, 128], bf16)
        nc.sync.dma_start(out=wqkv_sb[:],
                          in_=wqkv.ap().rearrange("q d p m -> p q d m"))
        bqk_sb = wp.tile([128, 2], f32)
        nc.sync.dma_start(out=bqk_sb[:], in_=bqk.ap().rearrange("q p -> p q"))
        bv_sb = wp.tile([64, 2], f32)
        nc.sync.dma_start(out=bv_sb[:], in_=bvp.ap())
        wos = wp.tile([128, D], bf16)
        ones64b = wp.tile([64, 1], bf16)
        nc.vector.memset(ones64b[:], 1.0)
        # row 64 of onesrow broadcasts the softmax-denominator reciprocals;
        # sel65 selects per-head scalars from partitions 0/64 (host-built:
        # partition-offset memsets fail the walrus ISA check)
        onesrow = wp.tile([65, 64], f32r)
        nc.sync.dma_start(out=onesrow[:], in_=onesrow_d.ap())
        sel65 = wp.tile([65, 128], f32)
        nc.sync.dma_start(out=sel65[:], in_=sel65_d.ap())

        # full x resident in SBUF (bf16): [d%128, d//128, token]
        xsb = wp.tile([128, 8, NT], bf16)
        for c8 in range(8):
            cols = slice(c8 * 512, (c8 + 1) * 512)
            nc.sync.dma_start(
                out=xsb[:, :, cols],
                in_=xbf.ap()[:, cols].rearrange("(dc p) t -> p dc t", p=128))
        # needed only at the projection stage — loaded after x
        wo_sb = wp.tile([128, D], bf16)
        nc.sync.dma_start(out=wo_sb[:], in_=wo.ap())

        QT = wp.tile([128, NT], bf16)
        KT = wp.tile([128, NT], bf16)
        # V' [tok-part, key-block, head, 66]; col 64 = ones, col 65 pad
        Vp = wp.tile([128, B * NJ, HPC, 66], bf16)
        ones_f = wp.tile([128, 1], f32)
        nc.vector.memset(ones_f[:], 1.0)
        nc.vector.tensor_copy(Vp[:, :, :, 64:65],
                              ones_f.broadcast_to([128, B * NJ, HPC, 1]))
        # packed per-head attention output (pre-norm), heads on partitions
        # packed per-head attention output (pre-norm): head h on partitions
        # h*64..h*64+63; head 1 lands there via SBUF->SBUF DMA from a stage
        osm = wp.tile([128, NT], bf16)
        # per-token squared norms, head h in row h*64 (aligned partitions)
        nsq = wp.tile([65, NT], f32)
        nc.vector.memset(nsq[:], 0.0)

        def qk_piece(c8, p, dst):
            cols = slice(c8 * 512, (c8 + 1) * 512)
            ps = psum.tile([128, 512], f32, tag="s2", name="ps_qk")
            for dc in range(8):
                nc.tensor.matmul(ps[:], wqkv_sb[:, p, dc, :],
                                 xsb[:, dc, cols],
                                 start=(dc == 0), stop=(dc == 7))
            nc.vector.tensor_scalar_add(dst[:, cols], ps[:], bqk_sb[:, p:p + 1])

        def v_piece(c8, t4):
            tok = slice(c8 * 512 + t4 * 128, c8 * 512 + (t4 + 1) * 128)
            pv = psum.tile([128, 2, 64], f32, tag="s2", name="ps_v")
            for dc in range(8):
                nc.tensor.matmul(pv[:], xsb[:, dc, tok],
                                 wqkv_sb[:, 2, dc, :],
                                 start=(dc == 0), stop=(dc == 7))
            nc.vector.tensor_copy(Vp[:, c8 * 4 + t4, :, 0:64], pv[:])

        def qkv_chunk(c8):
            """Project tokens [c8*512, (c8+1)*512) -> QT/KT cols, V' rows."""
            qk_piece(c8, 0, QT)
            qk_piece(c8, 1, KT)
            for t4 in range(4):
                v_piece(c8, t4)

        def qkv_inject(c8, j0=2):
            """Chunk c8 as j-indexed injection pieces for an attention loop."""
            return {j0: [lambda: qk_piece(c8, 0, QT)],
                    j0 + 1: [lambda: qk_piece(c8, 1, KT)],
                    j0 + 3: [lambda t4=t4: v_piece(c8, t4) for t4 in range(2)],
                    j0 + 4: [lambda t4=t4: v_piece(c8, t4)
                             for t4 in range(2, 4)]}

        def attn_qc(b, qc, pend_a=None, pend_b=None, inject=None):
            """One 512-query chunk of attention for batch b, both heads.

            Returns two closures with the normalization work: the caller
            re-injects them into the next chunk's j-loop (pend_a at j=1
            before av(0), pend_b at j=4) so slow cross-engine chains never
            sit at the head of the PE queue during the exp-bound stream.
            """
            qcols = slice(b * T + qc * 512, b * T + (qc + 1) * 512)
            po = [psum.tile([65, 512], f32, tag="po", bufs=2, name=f"po{h}")
                  for h in range(HPC)]
            prev_pe = None

            def av(j, pe):
                for h in range(HPC):
                    nc.tensor.matmul(po[h][:], Vp[:, b * NJ + j, h, 0:65],
                                     pe[:, h * 512:(h + 1) * 512],
                                     start=(j == 0), stop=(j == NJ - 1))

            for j in range(NJ):
                scols = slice(b * T + j * 128, b * T + (j + 1) * 128)
                s2 = psum.tile([128, 1024], f32, tag="s2", name="s2")
                for h in range(HPC):
                    nc.tensor.matmul(s2[:, h * 512:(h + 1) * 512],
                                     KT[h * 64:(h + 1) * 64, scols],
                                     QT[h * 64:(h + 1) * 64, qcols],
                                     start=True, stop=True,
                                     tile_position=(h * 64, 0))
                pe = pp.tile([128, 1024], bf16, tag="p")
                nc.scalar.activation(pe[:], s2[:], AF.Exp, scale=float(SCALE))
                if j == 1 and pend_a is not None:
                    # reads of the previous chunk's po slots must be emitted
                    # before av(0) rewrites them
                    pend_a()
                if j == 4 and pend_b is not None:
                    pend_b()
                if prev_pe is not None:
                    av(j - 1, prev_pe)
                if inject:
                    for fn in inject.get(j, ()):
                        fn()
                prev_pe = pe
            av(NJ - 1, prev_pe)

            qi = b * 4 + qc

            def deferred_a():
                # normalize by softmax denominators (row 64), add V bias.
                # The reciprocal reads the PSUM denominator row in place; a
                # PE matmul against a ones column broadcasts it across the
                # head's 64 partitions (staged to SBUF: engine ops may read
                # at most one PSUM operand, all operands partition-aligned).
                rstage = cc.tile([65, 2, 512], f32r, tag="dr")
                with nc.allow_low_precision(reason="f32r holds full f32 bits"):
                    for h in range(HPC):
                        nc.vector.reciprocal(rstage[64:65, h, :],
                                             po[h][64:65, :])
                bc = psum.tile([64, 2, 512], f32, tag="s2", name="bc")
                for h in range(HPC):
                    nc.tensor.matmul(bc[:, h, :], onesrow[64:65, :],
                                     rstage[64:65, h, :], start=True,
                                     stop=True, tile_position=(64, 0))
                bcs = cc.tile([64, 2, 512], f32, tag="bcs")
                nc.vector.tensor_copy(bcs[:], bc[:])
                stg = cc.tile([64, 512], bf16, tag="stg")
                dsts = [osm[0:64, qcols], stg[:]]
                for h in range(HPC):
                    nc.vector.tensor_tensor(dsts[h], po[h][0:64, :],
                                            bcs[:, h, :], op=ALU.mult)
                    nc.vector.tensor_scalar_add(dsts[h], dsts[h],
                                                bv_sb[:, h:h + 1])
                sq = cc.tile([64, 2, 512], bf16, tag="sq", name="sq")
                deferred_a.sq = sq
                for h in range(HPC):
                    nc.vector.tensor_tensor(sq[:, h, :], dsts[h], dsts[h],
                                            op=ALU.mult)
                # head 1 crosses to partitions 64:128 via SBUF->SBUF DMA
                nc.sync.dma_start(out=osm[64:128, qcols], in_=stg[:])

            def deferred_b():
                pn = psum.tile([65, 512], f32, tag="s2", name="pn")
                for h in range(HPC):
                    nc.tensor.matmul(pn[h * 64:h * 64 + 1, :], ones64b[:],
                                     deferred_a.sq[:, h, :],
                                     start=True, stop=True,
                                     tile_position=(0, h * 64))
                for h in range(HPC):
                    nc.vector.tensor_copy(nsq[h * 64:h * 64 + 1, qcols],
                                          pn[h * 64:h * 64 + 1, :])

            return deferred_a, deferred_b

        def _emit_all():
            if parts == "qkv":
                for c8 in range(8):
                    qkv_chunk(c8)
                nc.sync.dma_start(out=outp.ap()[0:128, :], in_=QT[:, 0:1024])
                return

            def merge_inj(*ds):
                out = {}
                for d in ds:
                    for k, v in d.items():
                        out.setdefault(k, []).extend(v)
                return out

            if INTERLEAVE_QKV:
                # chunks 0/1 up front; the rest stream through the attention
                # j-loops (chunk c's keys are first read at scores(j = 4c))
                qkv_chunk(0)
                qkv_chunk(1)
                injs = [merge_inj(qkv_inject(2, 2), qkv_inject(3, 6)),
                        qkv_inject(4), qkv_inject(5), qkv_inject(6),
                        qkv_inject(7), None, None, None]
                pa = pb = None
                for i in range(8):
                    pa, pb = attn_qc(i // 4, i % 4, pa, pb, inject=injs[i])
                pa(), pb()
            else:
                for c8 in range(4):
                    qkv_chunk(c8)
                pa = pb = None
                for qc in range(4):
                    pa, pb = attn_qc(0, qc, pa, pb)
                for c8 in range(4, 8):
                    qkv_chunk(c8)
                for qc in range(4):
                    pa, pb = attn_qc(1, qc, pa, pb)
                pa(), pb()

            # ---- per-head scale folded into W_o rows ----
            tots = sc.tile([65, 1], f32, tag="sc2")
            nc.scalar.activation(nsq[:], nsq[:], AF.Sqrt, accum_out=tots[:])
            den = sc.tile([65, 1], f32, tag="sc2")
            nc.vector.tensor_scalar(den[:], tots[:], 1.0 / NT, 1e-5,
                                    op0=ALU.mult, op1=ALU.max)
            inv = sc.tile([65, 1], f32, tag="sc2")
            nc.vector.reciprocal(inv[:], den[:])
            ibc = psum.tile([128, 1], f32, tag="po", bufs=2, name="ibc")
            nc.tensor.matmul(ibc[:], sel65[:], inv[:], start=True, stop=True)
            inv128 = sc.tile([128, 1], f32, tag="sc128")
            nc.vector.tensor_copy(inv128[:], ibc[:])
            nc.vector.tensor_scalar(wos[:], wo_sb[:], inv128[:], None,
                                    op0=ALU.mult)
            if parts == "attn":
                nc.sync.dma_start(out=outp.ap()[0:128, :], in_=wos[:])
                return

            # ---- output projection, both heads in one matmul ----
            # 3 rotating PSUM slots; PSUM->SBUF copies alternate DVE / Act
            # (GPSIMD cannot read PSUM); one 512KB DMA per pair of row tiles.
            for t2 in range(NT // 256):
                osb = ob.tile([128, 2, D], bf16, tag="ob")
                for i in range(2):
                    t = t2 * 2 + i
                    trows = slice(t * 128, (t + 1) * 128)
                    ppj = psum.tile([128, D], f32, tag="s2", name="ppj")
                    for dh in range(2):
                        dcols = slice(dh * 512, (dh + 1) * 512)
                        nc.tensor.matmul(ppj[:, dcols], osm[:, trows],
                                         wos[:, dcols], start=True, stop=True)
                    if t % 2 == 0:
                        nc.scalar.activation(osb[:, i, :], ppj[:], AF.Copy)
                    else:
                        nc.vector.tensor_copy(osb[:, i, :], ppj[:])
                nc.sync.dma_start(
                    out=outp.ap()[t2 * 256:(t2 + 1) * 256, :]
                    .rearrange("(i p) d -> p i d", p=128), in_=osb[:])

            if debug:
                nc.sync.dma_start(out=dQT.ap(), in_=QT[:])
                nc.sync.dma_start(out=dKT.ap(), in_=KT[:])
                nc.sync.dma_start(out=dVp.ap(), in_=Vp[:])
                nc.sync.dma_start(out=dosm.ap(), in_=osm[:])
                nc.sync.dma_start(out=dnsq.ap(), in_=nsq[:])
                nc.sync.dma_start(out=dwos.ap(), in_=wos[:])

        if repeat > 1:
            with tc.For_i(0, repeat, 1):
                _emit_all()
        else:
            _emit_all()

    nc.compile()
    return nc


# ---------------------------------------------------------------------------
# masked fallback path (original implementation)
# ---------------------------------------------------------------------------

def _build_masked(repeat: int = 1):
    with_mask = True
    nc = bacc.Bacc(None, target_bir_lowering=False)

    xT = nc.declare_dram_parameter("xT", [D, NT], f32r, isOutput=False)
    wqkv = nc.declare_dram_parameter("wqkv", [3, 8, 128, 128], f32r, isOutput=False)
    bqk = nc.declare_dram_parameter("bqk", [2, 128], f32, isOutput=False)
    bv = nc.declare_dram_parameter("bv", [HPC, HD], f32, isOutput=False)
    wo = nc.declare_dram_parameter("wo", [HPC, HD, D], f32r, isOutput=False)
    outp = nc.declare_dram_parameter("outp", [NT, D], f32, isOutput=True)
    if with_mask:
        maskT = nc.declare_dram_parameter("maskT", [T, T], f32, isOutput=False)

    with tile.TileContext(nc) as tc, contextlib.ExitStack() as ctx:
        wp = ctx.enter_context(tc.tile_pool(name="wp", bufs=1))
        big = ctx.enter_context(tc.tile_pool(name="big", bufs=2))
        xp = ctx.enter_context(tc.tile_pool(name="xp", bufs=3 if with_mask else 4))
        vtp = ctx.enter_context(tc.tile_pool(name="vtp", bufs=2))
        pp = ctx.enter_context(tc.tile_pool(name="pp", bufs=4))
        rowp = ctx.enter_context(tc.tile_pool(name="rowp", bufs=2))
        auxp = ctx.enter_context(tc.tile_pool(name="auxp", bufs=2))
        nsqp = ctx.enter_context(tc.tile_pool(name="nsqp", bufs=2))
        osmp = ctx.enter_context(tc.tile_pool(name="osmp", bufs=2))
        scp = ctx.enter_context(tc.tile_pool(name="scp", bufs=6))
        op = ctx.enter_context(tc.tile_pool(name="op", bufs=2))
        if with_mask:
            mp = ctx.enter_context(tc.tile_pool(name="mp", bufs=2))
        psum = ctx.enter_context(tc.tile_pool(name="psum", bufs=4, space="PSUM"))

        # ---- constants / weights ----
        wqkv_sb = wp.tile([128, 3, 8, 128], f32r)
        nc.sync.dma_start(out=wqkv_sb[:], in_=wqkv.ap().rearrange("q d p m -> p q d m"))
        bqk_sb = wp.tile([128, 2], f32)
        nc.sync.dma_start(out=bqk_sb[:], in_=bqk.ap().rearrange("q p -> p q"))
        bv_sb = wp.tile([HD, HPC], f32)
        nc.sync.dma_start(out=bv_sb[:], in_=bv.ap().rearrange("h p -> p h"))
        wo_sb = wp.tile([HD, HPC, D], f32r)
        nc.sync.dma_start(out=wo_sb[:], in_=wo.ap().rearrange("h p d -> p h d"))
        ones_f = wp.tile([128, 1], f32)
        nc.vector.memset(ones_f[:], 1.0)
        ones64 = wp.tile([HD, 1], f32r)
        nc.vector.tensor_copy(ones64[:], ones_f[0:64, :])
        ident = wp.tile([128, 128], f32)
        make_identity(nc, ident[:])

        # V' [s-part, s-chunk, head, 66]: cols 0:64 = V, col 64 = ones, col 65 pad
        Vp = wp.tile([128, NT // 128, HPC, 66], f32r)
        nc.vector.tensor_copy(Vp[:, :, :, 64:65],
                              ones_f.broadcast_to([128, NT // 128, HPC, 1]))

        QT = big.tile([128, NT], f32r, tag="big")
        KT = big.tile([128, NT], f32r, tag="big")

        # ---- per-head state for phase C ----
        osm = [None, None]       # out_sm [64, NT] fp32
        nsq = [None, None]       # per-token squared norms [1, NT]
        for h in range(HPC):
            osm[h] = osmp.tile([HD, NT], f32, name=f"osm{h}", tag="osm")
            nsq[h] = nsqp.tile([1, NT], f32, name=f"nsq{h}", tag="nsq")

        def qkv_chunk(c8):
            """Project tokens [c8*512, (c8+1)*512) -> QT, KT cols; V' rows."""
            halves = []
            for hh in range(2):
                xs = xp.tile([128, 4, 512], f32r, tag="xslab", name=f"xs{hh}")
                # one DMA per 256KB d-chunk so transfers spread across queues
                for dd in range(4):
                    nc.sync.dma_start(
                        out=xs[:, dd, :],
                        in_=xT.ap()[:, c8 * 512:(c8 + 1) * 512]
                        .rearrange("(dc p) t -> p dc t", p=128)[:, hh * 4 + dd, :])
                halves.append(xs)
            def xsl(dc):
                return halves[dc // 4][:, dc % 4, :]
            cols = slice(c8 * 512, (c8 + 1) * 512)
            for p, dst in ((0, QT), (1, KT)):
                ps = psum.tile([128, 512], f32, tag="a", name="ps_qkv")
                for dc in range(8):
                    nc.tensor.matmul(ps[:], wqkv_sb[:, p, dc, :], xsl(dc),
                                     start=(dc == 0), stop=(dc == 7))
                # rounds to f32r on write; adds per-partition bias
                nc.vector.tensor_scalar_add(dst[:, cols], ps[:], bqk_sb[:, p:p + 1])
            # V projection -> VT chunk [128(hd2), 512]
            psv = psum.tile([128, 512], f32, tag="a", name="ps_v")
            for dc in range(8):
                nc.tensor.matmul(psv[:], wqkv_sb[:, 2, dc, :], xsl(dc),
                                 start=(dc == 0), stop=(dc == 7))
            vt = vtp.tile([128, 512], f32, tag="vt")
            nc.scalar.activation(vt[:], psv[:], AF.Copy)
            # transpose VT -> V' (per head, 4 s-tiles of 128)
            for s4 in range(4):
                j = c8 * 4 + s4
                for h in range(HPC):
                    pt = psum.tile([128, 64], f32, tag="a", name="ps_tr")
                    nc.tensor.transpose(
                        pt[:], vt[h * 64:(h + 1) * 64, s4 * 128:(s4 + 1) * 128],
                        ident[h * 64:(h + 1) * 64, h * 64:(h + 1) * 64])
                    nc.vector.tensor_copy(Vp[:, j, h, 0:64], pt[:])

        def attn_qc(b, qc):
            """One 512-query chunk of attention for batch b, both heads."""
            qcols = slice(b * T + qc * 512, b * T + (qc + 1) * 512)
            po = [psum.tile([65, 512], f32, tag="a", name=f"po{h}") for h in range(HPC)]
            NJ = T // 128
            prev_pe = None

            def av(j, pe):
                for h in range(HPC):
                    nc.tensor.matmul(po[h][:], Vp[:, b * NJ + j, h, 0:65],
                                     pe[:, h * 512:(h + 1) * 512],
                                     start=(j == 0), stop=(j == NJ - 1))

            for j in range(NJ):
                scols = slice(b * T + j * 128, b * T + (j + 1) * 128)
                s2 = psum.tile([128, 1024], f32, tag="s2", bufs=2, name="s2")
                for h in range(HPC):
                    nc.tensor.matmul(s2[:, h * 512:(h + 1) * 512],
                                     KT[h * 64:(h + 1) * 64, scols],
                                     QT[h * 64:(h + 1) * 64, qcols],
                                     start=True, stop=True,
                                     tile_position=(h * 64, 0))
                pe = pp.tile([128, 1024], f32r, tag="p")
                if with_mask:
                    mt = mp.tile([128, 512], f32, tag="m")
                    nc.sync.dma_start(
                        out=mt[:],
                        in_=maskT.ap()[j * 128:(j + 1) * 128,
                                       qc * 512:(qc + 1) * 512])
                    tmp = pp.tile([128, 1024], f32, tag="tmp", bufs=2)
                    for h in range(HPC):
                        nc.vector.scalar_tensor_tensor(
                            tmp[:, h * 512:(h + 1) * 512],
                            s2[:, h * 512:(h + 1) * 512], SCALE, mt[:],
                            op0=ALU.mult, op1=ALU.add)
                    nc.scalar.activation(pe[:], tmp[:], AF.Exp)
                else:
                    nc.scalar.activation(pe[:], s2[:], AF.Exp, scale=float(SCALE))
                if prev_pe is not None:
                    av(j - 1, prev_pe)
                prev_pe = pe
            av(NJ - 1, prev_pe)
            # phase-C chunk work, inline: normalize by softmax denom, add bv
            for h in range(HPC):
                o65 = auxp.tile([65, 512], f32, tag="o65", bufs=2)
                nc.vector.tensor_copy(o65[:], po[h][:])
                drow = rowp.tile([1, 512], f32, tag="row")
                nc.sync.dma_start(out=drow[:], in_=o65[64:65, :])
                rrow = rowp.tile([1, 512], f32, tag="row")
                nc.vector.reciprocal(rrow[:], drow[:])
                bc = auxp.tile([HD, 512], f32, tag="aux")
                nc.gpsimd.partition_broadcast(bc[:], rrow[:])
                t1 = auxp.tile([HD, 512], f32, tag="aux")
                nc.vector.tensor_tensor(t1[:], o65[0:64, :], bc[:], op=ALU.mult)
                oc = osm[h][:, qcols]
                nc.vector.tensor_scalar_add(oc, t1[:], bv_sb[:, h:h + 1])
                # running norm sums: sq -> column sums (PE) -> sqrt -> row sum
                sq = auxp.tile([HD, 512], f32r, tag="aux")
                nc.vector.tensor_tensor(sq[:], oc, oc, op=ALU.mult)
                pn = psum.tile([1, 512], f32, tag="a", name="ps_n")
                nc.tensor.matmul(pn[:], ones64[:], sq[:], start=True, stop=True)
                nc.vector.tensor_copy(nsq[h][:, qcols], pn[:])

        def _emit_all():
            for c8 in range(4):
                qkv_chunk(c8)
            for qc in range(4):
                attn_qc(0, qc)
            for c8 in range(4, 8):
                qkv_chunk(c8)
            for qc in range(4):
                attn_qc(1, qc)

            # ---- finalize per-head scale, apply, project ----
            onn = [None, None]
            for h in range(HPC):
                onn[h] = big.tile([HD, NT], f32r, tag="big", name=f"onn{h}")
                tot = scp.tile([1, 1], f32, tag="sc")
                nc.scalar.activation(onn[h][0:1, :], nsq[h][:], AF.Sqrt,
                                     accum_out=tot[:])
                den = scp.tile([1, 1], f32, tag="sc")
                nc.vector.tensor_scalar(den[:], tot[:], 1.0 / NT, 1e-5,
                                        op0=ALU.mult, op1=ALU.max)
                inv = scp.tile([1, 1], f32, tag="sc")
                nc.vector.reciprocal(inv[:], den[:])
                inv64 = scp.tile([HD, 1], f32, tag="sc64")
                nc.gpsimd.partition_broadcast(inv64[:], inv[:])
                nc.vector.tensor_scalar(onn[h][:], osm[h][:], inv64[:], None, op0=ALU.mult)

            for t in range(NT // 128):
                trows = slice(t * 128, (t + 1) * 128)
                for dchunk in range(2):
                    dcols = slice(dchunk * 512, (dchunk + 1) * 512)
                    ppj = psum.tile([128, 512], f32, tag="a", name="ps_p")
                    for h in range(HPC):
                        nc.tensor.matmul(ppj[:], onn[h][:, trows], wo_sb[:, h, dcols],
                                         start=(h == 0), stop=(h == HPC - 1))
                    osb = op.tile([128, 512], f32, tag="ob")
                    nc.vector.tensor_copy(osb[:], ppj[:])
                    nc.sync.dma_start(out=outp.ap()[trows, dcols], in_=osb[:])

        if repeat > 1:
            with tc.For_i(0, repeat, 1):
                _emit_all()
        else:
            _emit_all()

    nc.compile()
    return nc


def _get_nc(with_mask: bool):
    key = with_mask
    if key not in _BUILD_CACHE:
        _BUILD_CACHE[key] = _build(with_mask)
    return _BUILD_CACHE[key]


def _prep_in_maps(inputs: dict, with_mask: bool):
    hidden_states = np.asarray(inputs["hidden_states"], dtype=np.float32)
    attn_mask = np.asarray(inputs["attn_mask"], dtype=np.float32)
    W_q, b_q = np.asarray(inputs["W_q"], np.float32), np.asarray(inputs["b_q"], np.float32)
    W_k, b_k = np.asarray(inputs["W_k"], np.float32), np.asarray(inputs["b_k"], np.float32)
    W_v, b_v = np.asarray(inputs["W_v"], np.float32), np.asarray(inputs["b_v"], np.float32)
    W_o, b_o = np.asarray(inputs["W_o"], np.float32), np.asarray(inputs["b_o"], np.float32)
    gate = np.asarray(inputs["gate"], np.float32)

    x = hidden_states.reshape(NT, D)
    xT = np.ascontiguousarray(x.T)
    g = np.clip(gate, 0.0, 1.0)

    in_maps = []
    if not with_mask:
        xT_bf = np.ascontiguousarray(xT.astype(ml_dtypes.bfloat16))
        for c in range(NCORES):
            hs = slice(c * HPC, (c + 1) * HPC)
            wq = np.concatenate([W_q[c * HPC + i] for i in range(HPC)], axis=1)
            wk = np.concatenate([W_k[c * HPC + i] for i in range(HPC)], axis=1)
            wv = np.concatenate([W_v[c * HPC + i] for i in range(HPC)], axis=1)
            wqkv_c = np.ascontiguousarray(
                np.stack([wq, wk, wv], axis=0).reshape(3, 8, 128, 128)
                .astype(ml_dtypes.bfloat16))
            bqk_c = np.ascontiguousarray(np.stack(
                [np.concatenate([b_q[c * HPC + i] for i in range(HPC)]),
                 np.concatenate([b_k[c * HPC + i] for i in range(HPC)])], axis=0))
            bv_c = np.ascontiguousarray(
                np.stack([b_v[c * HPC + i] for i in range(HPC)], axis=1))
            wo_c = np.ascontiguousarray(
                np.concatenate(
                    [W_o[c * HPC + i] * (g[c * HPC + i] / H) for i in range(HPC)],
                    axis=0).astype(ml_dtypes.bfloat16))      # [128, D]
            onesrow_np = np.zeros((65, 64), dtype=np.float32)
            onesrow_np[64, :] = 1.0
            sel65_np = np.zeros((65, 128), dtype=np.float32)
            sel65_np[0, 0:64] = 1.0
            sel65_np[64, 64:128] = 1.0
            in_maps.append(dict(xbf=xT_bf, wqkv=wqkv_c, bqk=bqk_c, bvp=bv_c,
                                wo=wo_c, onesrow_d=onesrow_np,
                                sel65_d=sel65_np))
        return in_maps

    for c in range(NCORES):
        hs = slice(c * HPC, (c + 1) * HPC)
        wq = np.concatenate([W_q[c * HPC + i] for i in range(HPC)], axis=1)
        wk = np.concatenate([W_k[c * HPC + i] for i in range(HPC)], axis=1)
        wv = np.concatenate([W_v[c * HPC + i] for i in range(HPC)], axis=1)
        wqkv_c = np.ascontiguousarray(
            np.stack([wq, wk, wv], axis=0).reshape(3, 8, 128, 128))
        bqk_c = np.ascontiguousarray(np.stack(
            [np.concatenate([b_q[c * HPC + i] for i in range(HPC)]),
             np.concatenate([b_k[c * HPC + i] for i in range(HPC)])], axis=0))
        bv_c = np.ascontiguousarray(b_v[hs])                      # [2, 64]
        wo_c = np.ascontiguousarray(
            W_o[hs] * (g[hs, None, None] / H))                    # [2, 64, D]
        m = dict(xT=xT, wqkv=wqkv_c, bqk=bqk_c, bv=bv_c, wo=wo_c)
        m["maskT"] = np.ascontiguousarray(attn_mask.T)
        in_maps.append(m)
    return in_maps


def kernel(hidden_states, attn_mask, W_q, b_q, W_k, b_k, W_v, b_v, W_o, b_o, gate):
    inputs = dict(hidden_states=hidden_states, attn_mask=attn_mask,
                  W_q=W_q, b_q=b_q, W_k=W_k, b_k=b_k, W_v=W_v, b_v=b_v,
                  W_o=W_o, b_o=b_o, gate=gate)
    attn_mask = np.asarray(attn_mask, dtype=np.float32)
    with_mask = bool(np.any(attn_mask))
    nc = _get_nc(with_mask)
    in_maps = _prep_in_maps(inputs, with_mask)

    res = run_bass_kernel_spmd(nc, in_maps, core_ids=list(range(NCORES)))
    if res.exec_time_ns is not None:
        print(f"HW exec time: {res.exec_time_ns} ns")

    out = np.zeros((NT, D), dtype=np.float32)
    for r in res.results:
        out += np.asarray(r["outp"], dtype=np.float32)
    gate = np.asarray(gate, np.float32)
    b_o = np.asarray(b_o, np.float32)
    b_eff = (np.clip(gate, 0.0, 1.0)[:, None] * b_o).sum(axis=0) / H
    out += b_eff[None, :]
    return out.reshape(B, T, D)

